# revision 2
# baseline (speedup 1.0000x reference)
"""Trainium2 kernel for nn_ContrastiveLoss_matrix (cross-attention i2t contrastive loss).

Strategy (8 NeuronCores, SPMD, caption-axis sharding):
  - Each core computes scores[:, m*16:(m+1)*16] (16 captions) for both the
    (im, s) and (pred, s_pred) terms.
  - Einsum 'brd,cwd->bcrw' is a [BR, D] @ [D, CW] matmul done in f32r
    (full-rate fp32 variant) with D on the contraction (partition) axis.
  - Word masking is pre-applied on the host by replacing padded caption
    words with a copy of word 0 (max over words is then exactly the max
    over valid words).
  - max over words: DVE segmented tensor_reduce over the PSUM tile viewed
    as [128, ncap, W].
  - sum over regions: PE matmul with a 0/1 indicator matrix A[br, b]
    (A.T @ mx accumulated over all row tiles).
  - divide by (obj_num + eps): per-partition scalar multiply by
    host-computed reciprocals.
  - Host gathers the 8 [128, 16] shards into scores [128, 128] and computes
    the (tiny) margin loss in numpy.
"""

import numpy as np

B = 128
R = 36
W = 50
Rp = 25
Wp = 30
D = 1024
P = 128
KO = D // P          # 8 contraction chunks
NCORES = 8
CSH = B // NCORES    # 16 captions per core
NEG = -1e30
EPS = 1e-6
MARGIN = 0.2

BR = B * R           # 4608  -> 36 row tiles
BRp = B * Rp         # 3200  -> 25 row tiles
NT1 = BR // P        # 36
NT2 = BRp // P       # 25
COLS1 = CSH * W      # 800
COLS2 = CSH * Wp     # 480
HALF1 = COLS1 // 2   # 400 (8 captions * 50 words per half)

_NC = None


def _build():
    import concourse.tile as tile
    from concourse import bacc, mybir

    f32 = mybir.dt.float32
    f32r = mybir.dt.float32r
    AX = mybir.AxisListType
    OP = mybir.AluOpType

    nc = bacc.Bacc("TRN2", target_bir_lowering=False, debug=False)

    imT = nc.dram_tensor("imT", [D, BR], f32r, kind="ExternalInput").ap()
    predT = nc.dram_tensor("predT", [D, BRp], f32r, kind="ExternalInput").ap()
    sT = nc.dram_tensor("sT", [D, COLS1], f32r, kind="ExternalInput").ap()
    spT = nc.dram_tensor("spT", [D, COLS2], f32r, kind="ExternalInput").ap()
    Aim = nc.dram_tensor("Aim", [BR, P], f32, kind="ExternalInput").ap()
    Apred = nc.dram_tensor("Apred", [BRp, P], f32, kind="ExternalInput").ap()
    inv_im = nc.dram_tensor("inv_im", [P, 1], f32, kind="ExternalInput").ap()
    inv_pred = nc.dram_tensor("inv_pred", [P, 1], f32, kind="ExternalInput").ap()
    out = nc.dram_tensor("scores_sh", [P, CSH], f32, kind="ExternalOutput").ap()

    imT_r = imT.rearrange("(ko p) n -> p ko n", p=P)      # [128, 8, 4608]
    predT_r = predT.rearrange("(ko p) n -> p ko n", p=P)  # [128, 8, 3200]
    sT_r = sT.rearrange("(ko p) n -> p ko n", p=P)        # [128, 8, 800]
    spT_r = spT.rearrange("(ko p) n -> p ko n", p=P)      # [128, 8, 480]
    Aim_r = Aim.rearrange("(t p) m -> t p m", p=P)        # [36, 128, 128]
    Apred_r = Apred.rearrange("(t p) m -> t p m", p=P)    # [25, 128, 128]

    with tile.TileContext(nc) as tc:
        with (
            tc.tile_pool(name="const", bufs=1) as const_pool,
            tc.tile_pool(name="imt", bufs=3) as im_pool,
            tc.tile_pool(name="at", bufs=3) as a_pool,
            tc.tile_pool(name="mx", bufs=4) as mx_pool,
            tc.tile_pool(name="fin", bufs=1) as fin_pool,
            tc.tile_pool(name="mm", bufs=4, space="PSUM") as psum_mm,
            tc.tile_pool(name="sim", bufs=2, space="PSUM") as psum_sim,
        ):
            s_sb = const_pool.tile([P, KO, COLS1], f32r)
            nc.sync.dma_start(s_sb[:], sT_r)
            sp_sb = const_pool.tile([P, KO, COLS2], f32r)
            nc.sync.dma_start(sp_sb[:], spT_r)
            inv_im_sb = const_pool.tile([P, 1], f32)
            nc.sync.dma_start(inv_im_sb[:], inv_im)
            inv_pred_sb = const_pool.tile([P, 1], f32)
            nc.sync.dma_start(inv_pred_sb[:], inv_pred)

            sim1 = psum_sim.tile([P, CSH], f32, tag="sim1")
            sim2 = psum_sim.tile([P, CSH], f32, tag="sim2")

            # ---- part 1: im x s ----
            for t in range(NT1):
                im_sb = im_pool.tile([P, KO, P], f32r, tag="imtile")
                nc.sync.dma_start(im_sb[:], imT_r[:, :, t * P:(t + 1) * P])
                a_sb = a_pool.tile([P, P], f32, tag="atile")
                nc.sync.dma_start(a_sb[:], Aim_r[t])
                mx = mx_pool.tile([P, CSH], f32, tag="mx")
                for h in range(2):
                    ps = psum_mm.tile([P, HALF1], f32, tag="mm")
                    for ko in range(KO):
                        nc.tensor.matmul(
                            ps[:],
                            im_sb[:, ko, :],
                            s_sb[:, ko, h * HALF1:(h + 1) * HALF1],
                            start=(ko == 0),
                            stop=(ko == KO - 1),
                        )
                    nc.vector.tensor_reduce(
                        out=mx[:, h * 8:(h + 1) * 8],
                        in_=ps.rearrange("p (c w) -> p c w", w=W),
                        axis=AX.X,
                        op=OP.max,
                    )
                nc.tensor.matmul(
                    sim1[:], a_sb[:], mx[:],
                    start=(t == 0), stop=(t == NT1 - 1),
                )

            # ---- part 2: pred x s_pred ----
            for t in range(NT2):
                im_sb = im_pool.tile([P, KO, P], f32r, tag="imtile")
                nc.sync.dma_start(im_sb[:], predT_r[:, :, t * P:(t + 1) * P])
                a_sb = a_pool.tile([P, P], f32, tag="atile")
                nc.sync.dma_start(a_sb[:], Apred_r[t])
                mx = mx_pool.tile([P, CSH], f32, tag="mx")
                ps = psum_mm.tile([P, COLS2], f32, tag="mm")
                for ko in range(KO):
                    nc.tensor.matmul(
                        ps[:],
                        im_sb[:, ko, :],
                        sp_sb[:, ko, :],
                        start=(ko == 0),
                        stop=(ko == KO - 1),
                    )
                nc.vector.tensor_reduce(
                    out=mx[:],
                    in_=ps.rearrange("p (c w) -> p c w", w=Wp),
                    axis=AX.X,
                    op=OP.max,
                )
                nc.tensor.matmul(
                    sim2[:], a_sb[:], mx[:],
                    start=(t == 0), stop=(t == NT2 - 1),
                )

            # ---- finalize: scores_sh = sim1*inv_im + sim2*inv_pred ----
            t1 = fin_pool.tile([P, CSH], f32)
            nc.vector.tensor_scalar_mul(t1[:], sim1[:], inv_im_sb[:])
            t2 = fin_pool.tile([P, CSH], f32)
            nc.vector.tensor_scalar_mul(t2[:], sim2[:], inv_pred_sb[:])
            o = fin_pool.tile([P, CSH], f32)
            nc.vector.tensor_add(o[:], t1[:], t2[:])
            nc.sync.dma_start(out, o[:])

    nc.compile()
    return nc


def _get_nc():
    global _NC
    if _NC is None:
        _NC = _build()
    return _NC


def _dup_pad_words(x, lens, width):
    # replace padded words with a copy of word 0 so that max over all words
    # == max over valid words (every row has >= 1 valid word)
    pad = np.arange(width)[None, :] >= lens[:, None]          # [B, W]
    return np.where(pad[:, :, None], x[:, :1, :], x)


LAST_RESULT = None


def kernel(im, im_l, s, s_l, pred, pred_l, s_pred, s_pred_l, _trace=False):
    from concourse.bass_utils import run_bass_kernel_spmd

    global LAST_RESULT
    im = np.asarray(im, np.float32)
    s = np.asarray(s, np.float32)
    pred = np.asarray(pred, np.float32)
    s_pred = np.asarray(s_pred, np.float32)
    im_l = np.asarray(im_l)
    s_l = np.asarray(s_l)
    pred_l = np.asarray(pred_l)
    s_pred_l = np.asarray(s_pred_l)

    s_fix = _dup_pad_words(s, s_l, W)
    sp_fix = _dup_pad_words(s_pred, s_pred_l, Wp)

    imT = np.ascontiguousarray(im.reshape(BR, D).T)
    predT = np.ascontiguousarray(pred.reshape(BRp, D).T)
    sT = np.ascontiguousarray(s_fix.reshape(B * W, D).T)      # [D, 6400]
    spT = np.ascontiguousarray(sp_fix.reshape(B * Wp, D).T)   # [D, 3840]

    Aim = np.zeros((BR, P), np.float32)
    Aim[np.arange(BR), np.arange(BR) // R] = 1.0
    Apred = np.zeros((BRp, P), np.float32)
    Apred[np.arange(BRp), np.arange(BRp) // Rp] = 1.0

    inv_im = (np.float32(1.0) / (im_l.astype(np.float32) + np.float32(EPS)))
    inv_pred = (np.float32(1.0) / (pred_l.astype(np.float32) + np.float32(EPS)))

    in_maps = []
    for m in range(NCORES):
        in_maps.append({
            "imT": imT,
            "predT": predT,
            "sT": np.ascontiguousarray(sT[:, m * COLS1:(m + 1) * COLS1]),
            "spT": np.ascontiguousarray(spT[:, m * COLS2:(m + 1) * COLS2]),
            "Aim": Aim,
            "Apred": Apred,
            "inv_im": inv_im.reshape(P, 1),
            "inv_pred": inv_pred.reshape(P, 1),
        })

    nc = _get_nc()
    res = run_bass_kernel_spmd(
        nc, in_maps, core_ids=list(range(NCORES)), trace=_trace,
        trace_cores=list(range(NCORES)) if _trace else None,
    )
    LAST_RESULT = res

    scores = np.concatenate(
        [res.results[m]["scores_sh"] for m in range(NCORES)], axis=1
    ).astype(np.float32)

    diag = np.diagonal(scores).copy()[:, None]                 # [B, 1]
    cost_s = np.clip(MARGIN + scores - diag, 0.0, None)
    cost_im = np.clip(MARGIN + scores - diag.T, 0.0, None)
    np.fill_diagonal(cost_s, 0.0)
    np.fill_diagonal(cost_im, 0.0)
    loss = np.float32(cost_s.sum(dtype=np.float32) + cost_im.sum(dtype=np.float32))
    return loss, scores


# revision 3
# speedup vs baseline: 1.0405x; 1.0405x over previous
"""Trainium2 kernel for nn_ContrastiveLoss_matrix (cross-attention i2t contrastive loss).

Strategy (8 NeuronCores, SPMD, caption-axis sharding):
  - Each core computes scores[:, m*16:(m+1)*16] (16 captions) for both the
    (im, s) and (pred, s_pred) terms.
  - Einsum 'brd,cwd->bcrw' is a [BR, D] @ [D, CW] matmul done in f32r
    (full-rate fp32 variant) with D on the contraction (partition) axis.
  - Word masking is pre-applied on the host by replacing padded caption
    words with a copy of word 0 (max over words is then exactly the max
    over valid words).
  - max over words: DVE segmented tensor_reduce over the PSUM tile viewed
    as [128, ncap, W].
  - sum over regions: PE matmul with a 0/1 indicator matrix A[br, b]
    (A.T @ mx accumulated over all row tiles), software-pipelined one
    iteration behind the main matmuls so the PE never waits on the DVE.
  - host applies the 1/(obj_num+eps) scaling, gathers the 8 shards into
    scores [128, 128], and computes the (tiny) margin loss in numpy.
"""

import numpy as np

B = 128
R = 36
W = 50
Rp = 25
Wp = 30
D = 1024
P = 128
KO = D // P          # 8 contraction chunks
NCORES = 8
CSH = B // NCORES    # 16 captions per core
EPS = 1e-6
MARGIN = 0.2

BR = B * R           # 4608  -> 36 row tiles
BRp = B * Rp         # 3200  -> 25 row tiles
NT1 = BR // P        # 36
NT2 = BRp // P       # 25
COLS1 = CSH * W      # 800
COLS2 = CSH * Wp     # 480
HALF1 = COLS1 // 2   # 400 (8 captions * 50 words per half)

_NC = None


def _build():
    import concourse.tile as tile
    from concourse import bacc, mybir

    f32 = mybir.dt.float32
    f32r = mybir.dt.float32r
    AX = mybir.AxisListType
    OP = mybir.AluOpType

    nc = bacc.Bacc("TRN2", target_bir_lowering=False, debug=False)

    imT = nc.dram_tensor("imT", [D, BR], f32r, kind="ExternalInput").ap()
    predT = nc.dram_tensor("predT", [D, BRp], f32r, kind="ExternalInput").ap()
    sT = nc.dram_tensor("sT", [D, COLS1], f32r, kind="ExternalInput").ap()
    spT = nc.dram_tensor("spT", [D, COLS2], f32r, kind="ExternalInput").ap()
    Aim = nc.dram_tensor("Aim", [BR, P], f32, kind="ExternalInput").ap()
    Apred = nc.dram_tensor("Apred", [BRp, P], f32, kind="ExternalInput").ap()
    out = nc.dram_tensor("sims", [P, 2 * CSH], f32, kind="ExternalOutput").ap()

    imT_r = imT.rearrange("(ko p) n -> p ko n", p=P)      # [128, 8, 4608]
    predT_r = predT.rearrange("(ko p) n -> p ko n", p=P)  # [128, 8, 3200]
    sT_r = sT.rearrange("(ko p) n -> p ko n", p=P)        # [128, 8, 800]
    spT_r = spT.rearrange("(ko p) n -> p ko n", p=P)      # [128, 8, 480]
    Aim_r = Aim.rearrange("(t p) m -> t p m", p=P)        # [36, 128, 128]
    Apred_r = Apred.rearrange("(t p) m -> t p m", p=P)    # [25, 128, 128]

    with tile.TileContext(nc) as tc:
        with (
            tc.tile_pool(name="const", bufs=1) as const_pool,
            tc.tile_pool(name="imt", bufs=3) as im_pool,
            tc.tile_pool(name="at", bufs=3) as a_pool,
            tc.tile_pool(name="mx", bufs=4) as mx_pool,
            tc.tile_pool(name="fin", bufs=1) as fin_pool,
            tc.tile_pool(name="mm", bufs=4, space="PSUM") as psum_mm,
            tc.tile_pool(name="sim", bufs=2, space="PSUM") as psum_sim,
        ):
            # caption shards stay resident; load in ko-chunks so the first
            # matmuls don't wait on the full transfer
            s_sb = const_pool.tile([P, KO, COLS1], f32r)
            for ko in range(KO):
                nc.sync.dma_start(s_sb[:, ko, :], sT_r[:, ko, :])
            sp_sb = const_pool.tile([P, KO, COLS2], f32r)

            sim1 = psum_sim.tile([P, CSH], f32, tag="sim1")
            sim2 = psum_sim.tile([P, CSH], f32, tag="sim2")

            # ---- part 1: im x s ----
            pend = None  # (a_sb, mx) of previous row tile
            for t in range(NT1):
                im_sb = im_pool.tile([P, KO, P], f32r, tag="imtile")
                nc.sync.dma_start(im_sb[:], imT_r[:, :, t * P:(t + 1) * P])
                a_sb = a_pool.tile([P, P], f32, tag="atile")
                nc.sync.dma_start(a_sb[:], Aim_r[t])
                mx = mx_pool.tile([P, CSH], f32, tag="mx")
                for h in range(2):
                    ps = psum_mm.tile([P, HALF1], f32, tag="mm")
                    for ko in range(KO):
                        nc.tensor.matmul(
                            ps[:],
                            im_sb[:, ko, :],
                            s_sb[:, ko, h * HALF1:(h + 1) * HALF1],
                            start=(ko == 0),
                            stop=(ko == KO - 1),
                        )
                    nc.vector.tensor_reduce(
                        out=mx[:, h * 8:(h + 1) * 8],
                        in_=ps.rearrange("p (c w) -> p c w", w=W),
                        axis=AX.X,
                        op=OP.max,
                    )
                if pend is not None:
                    nc.tensor.matmul(sim1[:], pend[0][:], pend[1][:],
                                     start=(t == 1), stop=False)
                pend = (a_sb, mx)
            nc.tensor.matmul(sim1[:], pend[0][:], pend[1][:],
                             start=(NT1 == 1), stop=True)

            # ---- part 2: pred x s_pred ----
            for ko in range(KO):
                nc.sync.dma_start(sp_sb[:, ko, :], spT_r[:, ko, :])
            pend = None
            for t in range(NT2):
                im_sb = im_pool.tile([P, KO, P], f32r, tag="imtile")
                nc.sync.dma_start(im_sb[:], predT_r[:, :, t * P:(t + 1) * P])
                a_sb = a_pool.tile([P, P], f32, tag="atile")
                nc.sync.dma_start(a_sb[:], Apred_r[t])
                mx = mx_pool.tile([P, CSH], f32, tag="mx")
                ps = psum_mm.tile([P, COLS2], f32, tag="mm")
                for ko in range(KO):
                    nc.tensor.matmul(
                        ps[:],
                        im_sb[:, ko, :],
                        sp_sb[:, ko, :],
                        start=(ko == 0),
                        stop=(ko == KO - 1),
                    )
                nc.vector.tensor_reduce(
                    out=mx[:],
                    in_=ps.rearrange("p (c w) -> p c w", w=Wp),
                    axis=AX.X,
                    op=OP.max,
                )
                if pend is not None:
                    nc.tensor.matmul(sim2[:], pend[0][:], pend[1][:],
                                     start=(t == 1), stop=False)
                pend = (a_sb, mx)
            nc.tensor.matmul(sim2[:], pend[0][:], pend[1][:],
                             start=(NT2 == 1), stop=True)

            # ---- tail: copy the two accumulators out (scaling on host) ----
            o = fin_pool.tile([P, 2 * CSH], f32)
            nc.scalar.copy(o[:, :CSH], sim1[:])
            nc.scalar.copy(o[:, CSH:], sim2[:])
            nc.sync.dma_start(out, o[:])

    nc.compile()
    return nc


def _get_nc():
    global _NC
    if _NC is None:
        _NC = _build()
    return _NC


def _dup_pad_words(x, lens, width):
    # replace padded words with a copy of word 0 so that max over all words
    # == max over valid words (every row has >= 1 valid word)
    pad = np.arange(width)[None, :] >= lens[:, None]          # [B, W]
    return np.where(pad[:, :, None], x[:, :1, :], x)


LAST_RESULT = None


def kernel(im, im_l, s, s_l, pred, pred_l, s_pred, s_pred_l, _trace=False):
    from concourse.bass_utils import run_bass_kernel_spmd

    global LAST_RESULT
    im = np.asarray(im, np.float32)
    s = np.asarray(s, np.float32)
    pred = np.asarray(pred, np.float32)
    s_pred = np.asarray(s_pred, np.float32)
    im_l = np.asarray(im_l)
    s_l = np.asarray(s_l)
    pred_l = np.asarray(pred_l)
    s_pred_l = np.asarray(s_pred_l)

    s_fix = _dup_pad_words(s, s_l, W)
    sp_fix = _dup_pad_words(s_pred, s_pred_l, Wp)

    imT = np.ascontiguousarray(im.reshape(BR, D).T)
    predT = np.ascontiguousarray(pred.reshape(BRp, D).T)
    sT = np.ascontiguousarray(s_fix.reshape(B * W, D).T)      # [D, 6400]
    spT = np.ascontiguousarray(sp_fix.reshape(B * Wp, D).T)   # [D, 3840]

    Aim = np.zeros((BR, P), np.float32)
    Aim[np.arange(BR), np.arange(BR) // R] = 1.0
    Apred = np.zeros((BRp, P), np.float32)
    Apred[np.arange(BRp), np.arange(BRp) // Rp] = 1.0

    inv_im = (np.float32(1.0) / (im_l.astype(np.float32) + np.float32(EPS)))
    inv_pred = (np.float32(1.0) / (pred_l.astype(np.float32) + np.float32(EPS)))

    in_maps = []
    for m in range(NCORES):
        in_maps.append({
            "imT": imT,
            "predT": predT,
            "sT": np.ascontiguousarray(sT[:, m * COLS1:(m + 1) * COLS1]),
            "spT": np.ascontiguousarray(spT[:, m * COLS2:(m + 1) * COLS2]),
            "Aim": Aim,
            "Apred": Apred,
        })

    nc = _get_nc()
    res = run_bass_kernel_spmd(
        nc, in_maps, core_ids=list(range(NCORES)), trace=_trace,
        trace_cores=list(range(NCORES)) if _trace else None,
    )
    LAST_RESULT = res

    shards = []
    for m in range(NCORES):
        sims = res.results[m]["sims"]
        shard = sims[:, :CSH] * inv_im[:, None] + sims[:, CSH:] * inv_pred[:, None]
        shards.append(shard.astype(np.float32))
    scores = np.concatenate(shards, axis=1)

    diag = np.diagonal(scores).copy()[:, None]                 # [B, 1]
    cost_s = np.clip(MARGIN + scores - diag, 0.0, None)
    cost_im = np.clip(MARGIN + scores - diag.T, 0.0, None)
    np.fill_diagonal(cost_s, 0.0)
    np.fill_diagonal(cost_im, 0.0)
    loss = np.float32(cost_s.sum(dtype=np.float32) + cost_im.sum(dtype=np.float32))
    return loss, scores
